# revision 1
# baseline (speedup 1.0000x reference)
"""Distributed GQA attention kernel for 8 TRN2 NeuronCores.

Problem: B=2, S=2048, D=2048, 32 q-heads / 8 kv-heads, hd=64, causal + RoPE.

Strategy (sequence-sharded "context parallel"):
  - Each core owns 2 zigzag row-blocks per batch (blocks i and 15-i of 16),
    512 rows total. It computes Q for all 32 heads on its rows, K/V for all
    8 kv-heads on its rows, applies RoPE, then AllGathers K/V (about 1MB/rank,
    far cheaper than the 33MB AllReduce a head-sharded split would need).
  - Attention runs fully "transposed": projections produce qT/kT (head-dim on
    partitions) directly from x^T (host-pretransposed), scoresT = kT_tile.T @ qT
    come out with keys on partitions, probsT feeds P@V as the moving operand with
    V in natural layout as the stationary operand, and the PV output outT
    [hd, rows] is exactly the lhsT layout the output projection needs.
    No on-device transposes anywhere.
  - Softmax without max-subtraction (scores are bounded ~|4| for this data):
    probs = exp(s/8) * exp(mask), with the additive mask converted host-side to
    multiplicative per-tile factors (1/0 for causal). The denominator comes free
    from a ones-column appended to V (M=65 PV matmuls); normalization is applied
    to the attention output with a K=2 broadcast matmul + elementwise multiply.
  - Weight matrices are permuted host-side so that (a) RoPE's (even,odd) pairs
    are de-interleaved into [a(32)|b(32)] partition halves (RoPE becomes 3
    elementwise ops + partition-swap DMAs) and (b) q-heads pair up so 2 GQA
    groups pack the 128x128 PE array (K=64 row-group packing) in one shot.
  - Matmuls run in bf16 (1 cycle/row vs fp32's 4); psums/softmax stay fp32.

kernel(**inputs) -> np.ndarray  takes full inputs, returns full [2,2048,2048].
"""

import functools
import os
import sys
import types

import numpy as np
import ml_dtypes


class _StageDone(Exception):
    pass

BF16 = ml_dtypes.bfloat16

B, S, D = 2, 2048, 2048
NH, NKV, HD = 32, 8, 64
NREP = NH // NKV
NCORES = 8
BLK = 128
NBLK = S // BLK          # 16 blocks per batch
RPB = 2 * BLK            # rows per core per batch (2 blocks)
RT = B * RPB             # rows per core total = 512
KD = NKV * HD            # 512
VROW = 2 * HD + 2        # 130: [v_a | 1 | v_b | 1] per kv pair
CONTRIB_W = 4 * VROW     # 520


def _heads_of_tile(t):
    gg, m = divmod(t, 4)
    return 8 * gg + m, 8 * gg + 4 + m


def _core_blocks(i):
    return i, NBLK - 1 - i


# --------------------------------------------------------------------------
# device graph
# --------------------------------------------------------------------------

@functools.lru_cache(maxsize=None)
def _build_nc():
    import concourse.bacc as bacc
    import concourse.mybir as mybir
    import concourse.tile as tile

    BF = mybir.dt.bfloat16
    F32 = mybir.dt.float32
    EXP = mybir.ActivationFunctionType.Exp

    nc = bacc.Bacc(trn_type="TRN2", target_bir_lowering=False, debug=False,
                   num_devices=NCORES)

    xT_d = nc.declare_dram_parameter("xT", [D, RT], BF, isOutput=False)
    wq_d = nc.declare_dram_parameter("wq", [16, 16, 128, 128], BF, isOutput=False)
    wk_d = nc.declare_dram_parameter("wk", [16, 4, 128, 128], BF, isOutput=False)
    wv_d = nc.declare_dram_parameter("wv", [D, KD], BF, isOutput=False)
    wo_d = nc.declare_dram_parameter("wo", [D, D], BF, isOutput=False)
    crep_d = nc.declare_dram_parameter("crep", [128, RT], BF, isOutput=False)
    ssign_d = nc.declare_dram_parameter("ssign", [128, RT], BF, isOutput=False)
    mask_d = nc.declare_dram_parameter("maskm", [NBLK, 128, 512], BF, isOutput=False)
    out_d = nc.declare_dram_parameter("out", [RT, D], F32, isOutput=True)

    with tile.TileContext(nc) as tc:
        with tc.tile_pool(name="dram", bufs=1, space="DRAM") as dpool, \
             tc.tile_pool(name="const", bufs=1) as cpool, \
             tc.tile_pool(name="persist", bufs=1) as ppool, \
             tc.tile_pool(name="wstream", bufs=6) as wpool, \
             tc.tile_pool(name="work", bufs=3) as tpool, \
             tc.tile_pool(name="attn", bufs=3) as apool, \
             tc.tile_pool(name="ps", bufs=1, space="PSUM") as pspool:

            contrib = dpool.tile([2 * KD, CONTRIB_W], BF, name="contrib")
            gathered = dpool.tile([NCORES * 2 * KD, CONTRIB_W], BF,
                                  name="gathered", addr_space="Shared")

            # ---- constants ----
            crep = cpool.tile([128, RT], BF, name="crep", tag="crep")
            nc.sync.dma_start(out=crep[:, :], in_=crep_d[:, :])
            ssign = cpool.tile([128, RT], BF, name="ssign", tag="ssign")
            nc.sync.dma_start(out=ssign[:, :], in_=ssign_d[:, :])
            zt = cpool.tile([128, 512], BF, name="zt", tag="zt")
            nc.gpsimd.memset(zt[:, :], 0.0)
            msk = []
            for kb in range(NBLK):
                mt = cpool.tile([128, 512], BF, name=f"msk{kb}", tag=f"msk{kb}")
                nc.sync.dma_start(out=mt[:, :], in_=mask_d[kb, :, :])
                msk.append(mt)

            # ---- xT resident ----
            xt = []
            for k in range(16):
                t_ = ppool.tile([128, RT], BF, name=f"xt{k}", tag=f"xt{k}")
                nc.sync.dma_start(out=t_[:, :], in_=xT_d[k * 128:(k + 1) * 128, :])
                xt.append(t_)

            def rope(raw, out_t, out_halves=None):
                """raw [128, RT] bf16 (layout [a|b|a|b] x32) -> rotated+mixed.
                out_halves: optional pair of [64, RT] tiles to receive the two
                head halves at partition base 0 (avoids base-64 matmul operands,
                which fault the runtime)."""
                rot = tpool.tile([128, RT], BF, name="rot", tag="rot")
                for (db, sb) in ((0, 32), (32, 0), (64, 96), (96, 64)):
                    nc.gpsimd.dma_start(out=rot[db:db + 32, :],
                                        in_=raw[sb:sb + 32, :])
                t2 = tpool.tile([128, RT], BF, name="ropea", tag="ropea")
                t3 = tpool.tile([128, RT], BF, name="ropeb", tag="ropeb")
                nc.vector.tensor_mul(t2[:, :], raw[:, :], crep[:, :])
                nc.vector.tensor_mul(t3[:, :], rot[:, :], ssign[:, :])
                if out_halves is None:
                    nc.vector.tensor_add(out_t[:, :], t2[:, :], t3[:, :])
                else:
                    ha, hb = out_halves
                    nc.vector.tensor_add(ha[0:64, :], t2[0:64, :], t3[0:64, :])
                    nc.vector.tensor_add(hb[0:64, :], t2[64:128, :], t3[64:128, :])

            # ---- K projection + RoPE -> contrib ----
            kT = []
            for g in range(4):
                ps = pspool.tile([128, RT], F32, name=f"psk{g}", tag=f"pv{g % 4}")
                for kt in range(16):
                    wkt = wpool.tile([128, 128], BF, name="wkt", tag="wk")
                    (nc.sync if kt % 2 == 0 else nc.gpsimd).dma_start(
                        out=wkt[:, :], in_=wk_d[kt, g, :, :])
                    nc.tensor.matmul(ps[:, :], lhsT=wkt[:, :], rhs=xt[kt][:, :],
                                     start=(kt == 0), stop=(kt == 15))
                kraw = tpool.tile([128, RT], BF, name="kraw", tag="kraw")
                nc.vector.tensor_copy(out=kraw[:, :], in_=ps[:, :])
                kt_t = tpool.tile([128, RT], BF, name=f"kT{g}", tag="kTout")
                rope(kraw, kt_t)
                kT.append(kt_t)
                nc.sync.dma_start(out=contrib[g * 128:(g + 1) * 128, 0:RT],
                                  in_=kt_t[:, :])

            # ---- V projection -> contrib (with ones columns) ----
            for r in range(4):
                ps = pspool.tile([128, KD], F32, name=f"psv{r}", tag=f"pv{r % 4}")
                for kt in range(16):
                    wvt = wpool.tile([128, KD], BF, name="wvt", tag="wv")
                    (nc.sync if kt % 2 == 0 else nc.gpsimd).dma_start(
                        out=wvt[:, :], in_=wv_d[kt * 128:(kt + 1) * 128, :])
                    nc.tensor.matmul(ps[:, :], lhsT=xt[kt][:, r * 128:(r + 1) * 128],
                                     rhs=wvt[:, :], start=(kt == 0), stop=(kt == 15))
                vsb = tpool.tile([128, CONTRIB_W], BF, name="vsb", tag="vsb")
                vdst = vsb.rearrange("p (g t u) -> p g t u", g=4, t=2, u=VROW // 2)
                vsrc = ps.rearrange("p (g t u) -> p g t u", g=4, t=2, u=HD)
                nc.scalar.copy(out=vdst[:, :, :, 0:HD], in_=vsrc[:, :, :, :])
                nc.gpsimd.memset(vdst[:, :, :, HD:HD + 1], 1.0)
                nc.sync.dma_start(
                    out=contrib[KD + r * 128:KD + (r + 1) * 128, :],
                    in_=vsb[:, :])

            # ---- AllGather K/V ----
            nc.gpsimd.collective_compute(
                "AllGather", mybir.AluOpType.bypass,
                replica_groups=[list(range(NCORES))],
                ins=[contrib[:, :].opt()], outs=[gathered[:, :].opt()],
            )

            # ---- Q projection + RoPE (overlaps the AllGather) ----
            # qpa/qpb[gg][p]: [64, 1024] = cols [b0: m=2p | m=2p+1, b1: same],
            # a/b = first/second head of the GQA pair (kv 2gg / 2gg+1).
            qpa = [[None, None] for _ in range(4)]
            qpb = [[None, None] for _ in range(4)]
            for gg in range(4):
                for p in range(2):
                    qpa[gg][p] = ppool.tile([64, 1024], BF, name=f"qpa{gg}{p}",
                                            tag=f"qpa{gg}{p}")
                    qpb[gg][p] = ppool.tile([64, 1024], BF, name=f"qpb{gg}{p}",
                                            tag=f"qpb{gg}{p}")
            for t in range(16):
                gg, m = divmod(t, 4)
                p, half = divmod(m, 2)
                ps = pspool.tile([128, RT], F32, name=f"psq{t}", tag=f"pv{t % 4}")
                for kt in range(16):
                    wqt = wpool.tile([128, 128], BF, name="wqt", tag="wq")
                    (nc.sync if kt % 2 == 0 else nc.gpsimd).dma_start(
                        out=wqt[:, :], in_=wq_d[kt, t, :, :])
                    nc.tensor.matmul(ps[:, :], lhsT=wqt[:, :], rhs=xt[kt][:, :],
                                     start=(kt == 0), stop=(kt == 15))
                qraw = tpool.tile([128, RT], BF, name="qraw", tag="qraw")
                nc.vector.tensor_copy(out=qraw[:, :], in_=ps[:, :])
                rot = tpool.tile([128, RT], BF, name="rot", tag="rot")
                for (db, sb) in ((0, 32), (32, 0), (64, 96), (96, 64)):
                    nc.gpsimd.dma_start(out=rot[db:db + 32, :],
                                        in_=qraw[sb:sb + 32, :])
                t2 = tpool.tile([128, RT], BF, name="ropea", tag="ropea")
                t3 = tpool.tile([128, RT], BF, name="ropeb", tag="ropeb")
                nc.vector.tensor_mul(t2[:, :], qraw[:, :], crep[:, :])
                nc.vector.tensor_mul(t3[:, :], rot[:, :], ssign[:, :])
                for b_ in range(2):
                    d0 = b_ * 512 + half * 256
                    s0 = b_ * 256
                    nc.vector.tensor_add(qpa[gg][p][0:64, d0:d0 + 256],
                                         t2[0:64, s0:s0 + 256],
                                         t3[0:64, s0:s0 + 256])
                    nc.vector.tensor_add(qpb[gg][p][0:64, d0:d0 + 256],
                                         t2[64:128, s0:s0 + 256],
                                         t3[64:128, s0:s0 + 256])

            # ---- attention ----
            attnT = []
            for t in range(16):
                at = ppool.tile([128, RT], BF, name=f"attnT{t}", tag=f"attnT{t}")
                attnT.append(at)

            KEYS = (("a", 0), ("a", 1), ("b", 0), ("b", 1))
            for b in range(B):
                for gg in range(4):
                    pv = {}
                    for i_, key in enumerate(KEYS):
                        pv[key] = pspool.tile([65, 512], F32,
                                              name=f"pvb{i_}", tag=f"pv{i_}")
                    pending = []
                    for kb in range(NBLK):
                        r = kb if kb < 8 else 15 - kb
                        sslot = 0 if kb < 8 else 1
                        kof = b * RPB + sslot * 128
                        ksl_a = apool.tile([64, 128], BF, name="ksla", tag="ksla", bufs=6)
                        nc.sync.dma_start(
                            out=ksl_a[:, :],
                            in_=gathered[1024 * r + 128 * gg:
                                         1024 * r + 128 * gg + 64,
                                         kof:kof + 128])
                        ksl_b = apool.tile([64, 128], BF, name="kslb", tag="kslb", bufs=6)
                        nc.gpsimd.dma_start(
                            out=ksl_b[:, :],
                            in_=gathered[1024 * r + 128 * gg + 64:
                                         1024 * r + 128 * (gg + 1),
                                         kof:kof + 128])
                        vsl = apool.tile([128, VROW], BF, name="vsl", tag="vsl", bufs=8)
                        nc.sync.dma_start(
                            out=vsl[:, :],
                            in_=gathered[1024 * r + KD + kof:
                                         1024 * r + KD + kof + 128,
                                         VROW * gg:VROW * (gg + 1)])
                        cur = []
                        for half, ksl, qgrp, vcol in (
                                ("a", ksl_a, qpa[gg], 0),
                                ("b", ksl_b, qpb[gg], 65)):
                            for p in range(2):
                                sc = pspool.tile([128, 512], F32, name="sc",
                                                 tag="sc", bufs=4)
                                nc.tensor.matmul(
                                    sc[:, :], lhsT=ksl[:, :],
                                    rhs=qgrp[p][0:64, b * 512:b * 512 + 512],
                                    start=True, stop=True)
                                probs2 = apool.tile([128, 512], BF, name="probs2",
                                                    tag="probs2", bufs=10)
                                nc.scalar.activation(out=probs2[:, :], in_=sc[:, :],
                                                     func=EXP, scale=0.125)
                                pam2 = apool.tile([128, 512], BF, name="pam2",
                                                  tag="pam2", bufs=14)
                                nc.vector.tensor_mul(pam2[:, :], probs2[:, :],
                                                     msk[kb][:, :])
                                cur.append((half, p, vcol, pam2))
                        # PV matmuls run two kbs behind the scores so the PE
                        # never stalls on the exp/mask round-trip and the ACT
                        # always has a backlog of score tiles to exp.
                        pending.append((kb, vsl, cur))
                        if len(pending) > 3:
                            pkb, pvsl, plist = pending.pop(0)
                            for (half, p, vcol, pam2) in plist:
                                nc.tensor.matmul(
                                    pv[(half, p)][0:65, :],
                                    lhsT=pvsl[:, vcol:vcol + 65], rhs=pam2[:, :],
                                    start=(pkb == 0), stop=False)
                    for (pkb, pvsl, plist) in pending:
                        for (half, p, vcol, pam2) in plist:
                            nc.tensor.matmul(
                                pv[(half, p)][0:65, :],
                                lhsT=pvsl[:, vcol:vcol + 65], rhs=pam2[:, :],
                                start=(pkb == 0), stop=(pkb == NBLK - 1))

                    # ---- normalization ----
                    sums4 = apool.tile([128, 512], F32, name="sums4",
                                       tag="sums4", bufs=2)
                    for i_, key in enumerate(KEYS):
                        nc.vector.tensor_copy(out=sums4[32 * i_:32 * i_ + 1, :],
                                              in_=pv[key][64:65, :])
                    rec4 = apool.tile([128, 512], F32, name="rec4",
                                      tag="rec4", bufs=2)
                    nc.vector.reciprocal(out=rec4[:, :], in_=sums4[:, :])
                    for i_, (half, p) in enumerate(KEYS):
                        rec2 = apool.tile([1, 512], F32, name="rec2",
                                          tag="rec2", bufs=2)
                        # partition_broadcast reads physical partition 0 of its
                        # source tile (AP partition offsets are ignored), so
                        # stage each head-pair's row into a row-0 tile first.
                        nc.vector.tensor_copy(out=rec2[0:1, :],
                                              in_=rec4[32 * i_:32 * i_ + 1, :])
                        rep = apool.tile([128, 512], F32, name="repbc",
                                         tag="repbc", bufs=2)
                        nc.gpsimd.partition_broadcast(rep[:, :], rec2[0:1, :])
                        for mh in range(2):
                            t = 4 * gg + 2 * p + mh
                            qs = mh * 256
                            if half == "a":
                                nc.vector.tensor_mul(
                                    attnT[t][0:64, b * RPB:b * RPB + 256],
                                    pv[(half, p)][0:64, qs:qs + 256],
                                    rep[0:64, qs:qs + 256])
                            else:
                                nc.vector.tensor_mul(
                                    attnT[t][64:128, b * RPB:b * RPB + 256],
                                    pv[(half, p)][0:64, qs:qs + 256],
                                    rep[64:128, qs:qs + 256])


            # ---- output projection ----
            for dc in range(4):
                po = [pspool.tile([128, 512], F32, name=f"po{rt}", tag=f"pv{rt}")
                      for rt in range(4)]
                for t in range(16):
                    wot = wpool.tile([128, 512], BF, name="wot", tag="wo")
                    (nc.sync if t % 2 == 0 else nc.gpsimd).dma_start(
                        out=wot[:, :],
                        in_=wo_d[t * 128:(t + 1) * 128, dc * 512:(dc + 1) * 512])
                    for rt in range(4):
                        nc.tensor.matmul(po[rt][:, :],
                                         lhsT=attnT[t][:, rt * 128:(rt + 1) * 128],
                                         rhs=wot[:, :],
                                         start=(t == 0), stop=(t == 15))
                for rt in range(4):
                    ob = apool.tile([128, 512], F32, name="ob", tag="ob")
                    nc.vector.tensor_copy(out=ob[:, :], in_=po[rt][:, :])
                    nc.sync.dma_start(
                        out=out_d[rt * 128:(rt + 1) * 128,
                                  dc * 512:(dc + 1) * 512],
                        in_=ob[:, :])

    nc.compile()
    return nc


# --------------------------------------------------------------------------
# host-side sharding / layout prep
# --------------------------------------------------------------------------

def _prep_shared(wq, wk, wv, wo):
    qcol = np.zeros(D, np.int64)
    worow = np.zeros(D, np.int64)
    for t in range(16):
        ha, hb = _heads_of_tile(t)
        for half, h in enumerate((ha, hb)):
            base = t * 128 + half * 64
            qcol[base:base + 32] = h * 64 + np.arange(0, 64, 2)
            qcol[base + 32:base + 64] = h * 64 + np.arange(1, 64, 2)
            worow[base:base + 64] = h * 64 + np.arange(64)
    kcol = np.zeros(KD, np.int64)
    for g in range(NKV):
        base = g * 64
        kcol[base:base + 32] = g * 64 + np.arange(0, 64, 2)
        kcol[base + 32:base + 64] = g * 64 + np.arange(1, 64, 2)

    wq_t = wq[:, qcol].reshape(16, 128, 16, 128).transpose(0, 2, 1, 3)
    wq_t = np.ascontiguousarray(wq_t).astype(BF16)
    wk_t = wk[:, kcol].reshape(16, 128, 4, 128).transpose(0, 2, 1, 3)
    wk_t = np.ascontiguousarray(wk_t).astype(BF16)
    wv_c = np.ascontiguousarray(wv).astype(BF16)
    wo_c = np.ascontiguousarray(wo[worow, :]).astype(BF16)
    return wq_t, wk_t, wv_c, wo_c


def _prep_core(i, x, freqs_cos, freqs_sin, mask):
    bi, bj = _core_blocks(i)
    rows = np.concatenate([np.arange(bi * BLK, (bi + 1) * BLK),
                           np.arange(bj * BLK, (bj + 1) * BLK)])
    xs = np.concatenate([x[0, rows, :], x[1, rows, :]], axis=0)       # [512, D]
    xT = np.ascontiguousarray(xs.T).astype(BF16)                      # [D, 512]

    posf = np.concatenate([rows, rows])                               # [512]
    j = np.arange(128) % 32
    crep = freqs_cos[posf][:, j].T.astype(BF16)                       # [128, 512]
    sgn = np.where((np.arange(128) // 32) % 2 == 0, -1.0, 1.0).astype(np.float32)
    ssign = (freqs_sin[posf][:, j].T * sgn[:, None]).astype(BF16)

    maskm = np.zeros((NBLK, 128, 256), np.float32)
    for kb in range(NBLK):
        krows = mask[:, kb * BLK:(kb + 1) * BLK]                      # [S, 128]
        for col, blkq in enumerate((bi, bj)):
            madd = krows[blkq * BLK:(blkq + 1) * BLK, :]              # [128q,128k]
            maskm[kb][:, col * 128:(col + 1) * 128] = np.exp(madd.T)
    maskm = np.tile(maskm, (1, 1, 2)).astype(BF16)
    return xT, crep, ssign, maskm


def _assemble(results):
    out = np.empty((B, S, D), np.float32)
    for i in range(NCORES):
        bi, bj = _core_blocks(i)
        r = results[i]["out"]
        out[0, bi * BLK:(bi + 1) * BLK] = r[0:128]
        out[0, bj * BLK:(bj + 1) * BLK] = r[128:256]
        out[1, bi * BLK:(bi + 1) * BLK] = r[256:384]
        out[1, bj * BLK:(bj + 1) * BLK] = r[384:512]
    return out


LAST_RUN_INFO = {}


def kernel(x, freqs_cos, freqs_sin, mask, wq, wk, wv, wo, start_pos=0):
    from concourse.bass_utils import run_bass_kernel_spmd

    x = np.asarray(x, dtype=np.float32)
    freqs_cos = np.asarray(freqs_cos, dtype=np.float32)
    freqs_sin = np.asarray(freqs_sin, dtype=np.float32)
    mask = np.asarray(mask, dtype=np.float32)
    wq = np.asarray(wq, dtype=np.float32)
    wk = np.asarray(wk, dtype=np.float32)
    wv = np.asarray(wv, dtype=np.float32)
    wo = np.asarray(wo, dtype=np.float32)

    wq_t, wk_t, wv_c, wo_c = _prep_shared(wq, wk, wv, wo)
    in_maps = []
    for i in range(NCORES):
        xT, crep, ssign, maskm = _prep_core(i, x, freqs_cos, freqs_sin, mask)
        in_maps.append({
            "xT": xT, "wq": wq_t, "wk": wk_t, "wv": wv_c, "wo": wo_c,
            "crep": crep, "ssign": ssign, "maskm": maskm,
        })

    nc = _build_nc()

    trace = bool(int(os.environ.get("KERNEL_TRACE", "0")))
    kwargs = {}
    if trace:
        _install_ntff_hook()
        import concourse.bass_utils as bass_utils
        bass_utils.upload_artifacts = lambda tmpdir: tmpdir
        import tempfile
        tmpdir = tempfile.mkdtemp(prefix="attn_trace_")
        kwargs = {"trace": True, "tmpdir": tmpdir}

    res = run_bass_kernel_spmd(nc, in_maps, core_ids=list(range(NCORES)),
                               **kwargs)
    LAST_RUN_INFO.clear()
    LAST_RUN_INFO.update({
        "exec_time_ns": res.exec_time_ns,
        "tmpdir": kwargs.get("tmpdir"),
        "res": res,
    })
    return _assemble(res.results)


def _install_ntff_hook():
    if "antenv.axon_hooks" not in sys.modules:
        import antenv

        mod = types.ModuleType("antenv.axon_hooks")
        mod._hook = None
        mod.set_axon_ntff_profile_hook = lambda h: setattr(mod, "_hook", h)
        mod.get_axon_ntff_profile_hook = lambda: mod._hook
        sys.modules["antenv.axon_hooks"] = mod
        antenv.axon_hooks = mod
    from trn_agent_boot.trn_boot import _ntff_profile_via_ctypes
    from antenv.axon_hooks import set_axon_ntff_profile_hook as _set

    _set(_ntff_profile_via_ctypes("/opt/axon/libaxon_pjrt.so"))



# revision 5
# speedup vs baseline: 1.0855x; 1.0855x over previous
"""Distributed GQA attention kernel for 8 TRN2 NeuronCores.

Problem: B=2, S=2048, D=2048, 32 q-heads / 8 kv-heads, hd=64, causal + RoPE.

Strategy (kv-head tensor parallel, zero collectives):
  - Core c owns kv-head c (q-heads 4c..4c+3) for BOTH batches over ALL rows.
    Every core loads the full x (host-pretransposed to xT bf16) and projects
    Q (4 heads), K, V (1 kv head each) for all 4096 rows. K/V never leave the
    core, so there are NO collectives. Each core computes a PARTIAL output
    (its 4 heads x its 256 wo rows) and the host sums the 8 partials.
  - Because every core sees all rows, the causal structure is IDENTICAL on
    all cores (SPMD-compatible): per q-block qb only key blocks kb <= qb are
    computed -> ~47% of score/exp/PV work skipped exactly, with all matmuls
    staying N=512 wide (4 q-heads x 128 rows share one kv head -> one ksl
    stationary serves 4 heads; one vsl serves all q-blocks at a kb).
  - Attention runs fully "transposed": scoresT = ksl.T @ qT4 with keys on
    partitions; exp is merged into [128,1024] two-bank PSUM reads (one ACT
    instruction per 2 score tiles); only the diagonal tile gets a mask
    multiply. PV uses V in natural layout (via DMA-transpose from the
    projection) with a ones-column appended for the softmax denominator.
  - Softmax without max-subtraction: probs = exp(s/8); denominator from the
    ones-column; normalization applied to the PV output via
    reciprocal + partition_broadcast + elementwise multiply.
  - Matmuls in bf16; psums/softmax in fp32; partial output stored bf16.

kernel(**inputs) -> np.ndarray  takes full inputs, returns full [2,2048,2048].
"""

import functools
import os
import sys
import types

import numpy as np
import ml_dtypes

BF16 = ml_dtypes.bfloat16

B, S, D = 2, 2048, 2048
NH, NKV, HD = 32, 8, 64
BS = B * S               # 4096 rows total (b-major)
NB = S // 128            # 16 blocks per batch
NCORES = 8
VROW = HD + 1            # 65: [v | 1]


# --------------------------------------------------------------------------
# device graph (identical on all cores; per-core weights via input data)
# --------------------------------------------------------------------------

@functools.lru_cache(maxsize=None)
def _build_nc():
    import concourse.bacc as bacc
    import concourse.mybir as mybir
    import concourse.tile as tile

    BF = mybir.dt.bfloat16
    F32 = mybir.dt.float32
    EXP = mybir.ActivationFunctionType.Exp

    nc = bacc.Bacc(trn_type="TRN2", target_bir_lowering=False, debug=False,
                   num_devices=NCORES)

    dbg = bool(int(os.environ.get("KERNEL_DEBUG", "0")))
    if dbg:
        kT_dbg = nc.declare_dram_parameter("kT_dbg", [64, BS], BF,
                                           isOutput=True)
        vO_dbg = nc.declare_dram_parameter("vO_dbg", [128, 32 * 128], BF,
                                           isOutput=True)
        qT_dbg = nc.declare_dram_parameter("qT_dbg", [64, 32 * 512], BF,
                                           isOutput=True)
        aT_dbg = nc.declare_dram_parameter("aT_dbg", [2, 128, BS], BF,
                                           isOutput=True)

    xT_d = nc.declare_dram_parameter("xT", [D, BS], BF, isOutput=False)
    wq_d = nc.declare_dram_parameter("wq", [16, 2, 128, 128], BF, isOutput=False)
    wkv_d = nc.declare_dram_parameter("wkv", [16, 128, 128], BF, isOutput=False)
    wo_d = nc.declare_dram_parameter("wo", [2, 128, D], BF, isOutput=False)
    crep_d = nc.declare_dram_parameter("crep", [128, BS], BF, isOutput=False)
    ssig_d = nc.declare_dram_parameter("ssig", [128, BS], BF, isOutput=False)
    dmask_d = nc.declare_dram_parameter("dmask", [128, 512], BF, isOutput=False)
    out_d = nc.declare_dram_parameter("out", [BS, D], BF, isOutput=True)

    with tile.TileContext(nc) as tc:
        with tc.tile_pool(name="const", bufs=1) as cpool, \
             tc.tile_pool(name="persist", bufs=1) as ppool, \
             tc.tile_pool(name="xstream", bufs=32) as xpool, \
             tc.tile_pool(name="wstream", bufs=4) as wpool, \
             tc.tile_pool(name="work", bufs=3) as tpool, \
             tc.tile_pool(name="attn", bufs=3) as apool, \
             tc.tile_pool(name="ps", bufs=1, space="PSUM") as pspool:

            # ---- constants ----
            crep = cpool.tile([128, BS], BF, name="crep", tag="crep")
            nc.sync.dma_start(out=crep[:, :], in_=crep_d[:, :])
            ssig = cpool.tile([128, BS], BF, name="ssig", tag="ssig")
            nc.sync.dma_start(out=ssig[:, :], in_=ssig_d[:, :])
            dmask = cpool.tile([128, 512], BF, name="dmask", tag="dmask")
            nc.sync.dma_start(out=dmask[:, :], in_=dmask_d[:, :])
            wot = []
            for t in range(2):
                w_ = cpool.tile([128, D], BF, name=f"wot{t}", tag=f"wot{t}")
                nc.gpsimd.dma_start(out=w_[:, :], in_=wo_d[t, :, :])
                wot.append(w_)

            # ---- persistent activations ----
            kT = ppool.tile([64, BS], BF, name="kT", tag="kT")
            # V blocks padded to 128-col stride: dma_start_transpose needs
            # 128-byte-aligned output offsets (65-col stride corrupts).
            vOnes = ppool.tile([128, 32 * 128], BF, name="vOnes", tag="vOnes")
            vview = vOnes.rearrange("p (n w) -> p n w", w=128)
            nc.gpsimd.memset(vview[:, :, HD:HD + 1], 1.0)
            # qT4 cols: (b*16 + qb)*512 + h*128 + r   (h = head 0..3 local)
            qT4 = ppool.tile([64, 32 * 512], BF, name="qT4", tag="qT4")
            attnT = []
            for t in range(2):
                a_ = ppool.tile([128, BS], BF, name=f"attnT{t}", tag=f"attnT{t}")
                attnT.append(a_)

            # ---- projections (8 chunks of 512 rows; c = b*4 + g) ----
            for c in range(8):
                b, g = divmod(c, 4)
                cs = c * 512
                xt = []
                for k in range(16):
                    t_ = xpool.tile([128, 512], BF, name="xt", tag="xt")
                    eng = nc.sync if k % 2 == 0 else nc.scalar
                    eng.dma_start(out=t_[:, :],
                                  in_=xT_d[k * 128:(k + 1) * 128, cs:cs + 512])
                    xt.append(t_)

                # K|V fused projection: psum rows 0:64 = kT, 64:128 = vT
                ps = pspool.tile([128, 512], F32, name="pskv", tag=f"pv{c % 4}")
                for k in range(16):
                    wkv = wpool.tile([128, 128], BF, name="wkv", tag="wkv")
                    nc.gpsimd.dma_start(out=wkv[:, :], in_=wkv_d[k, :, :])
                    nc.tensor.matmul(ps[:, :], lhsT=wkv[:, :], rhs=xt[k][:, :],
                                     start=(k == 0), stop=(k == 15))
                kvraw = tpool.tile([128, 512], BF, name="kvraw", tag="kvraw")
                nc.vector.tensor_copy(out=kvraw[:, :], in_=ps[:, :])
                # K rope (rows 0:64; [ev32|od32] de-interleaved)
                rot = tpool.tile([128, 512], BF, name="rot", tag="rot")
                nc.gpsimd.dma_start(out=rot[0:32, :], in_=kvraw[32:64, :])
                nc.gpsimd.dma_start(out=rot[32:64, :], in_=kvraw[0:32, :])
                t2 = tpool.tile([128, 512], BF, name="t2", tag="t2")
                t3 = tpool.tile([128, 512], BF, name="t3", tag="t3")
                nc.vector.tensor_mul(t2[0:64, :], kvraw[0:64, :],
                                     crep[0:64, cs:cs + 512])
                nc.vector.tensor_mul(t3[0:64, :], rot[0:64, :],
                                     ssig[0:64, cs:cs + 512])
                nc.vector.tensor_add(kT[0:64, cs:cs + 512],
                                     t2[0:64, :], t3[0:64, :])
                # V natural layout via DMA transpose (rows 64:128 of kvraw)
                for j in range(4):
                    blk = c * 4 + j      # global (b*16 + kb)
                    nc.sync.dma_start_transpose(
                        out=vOnes[:, blk * 128:blk * 128 + HD],
                        in_=kvraw[64:128, j * 128:(j + 1) * 128])

                # Q projection + rope -> qT4
                for hp in range(2):
                    psq = pspool.tile([128, 512], F32, name="psq",
                                      tag=f"pv{(c + 2 + hp) % 4}")
                    for k in range(16):
                        wq = wpool.tile([128, 128], BF, name="wq", tag="wq")
                        nc.gpsimd.dma_start(out=wq[:, :], in_=wq_d[k, hp, :, :])
                        nc.tensor.matmul(psq[:, :], lhsT=wq[:, :],
                                         rhs=xt[k][:, :],
                                         start=(k == 0), stop=(k == 15))
                    qraw = tpool.tile([128, 512], BF, name="qraw", tag="qraw")
                    nc.vector.tensor_copy(out=qraw[:, :], in_=psq[:, :])
                    rotq = tpool.tile([128, 512], BF, name="rotq", tag="rotq")
                    for (db, sb) in ((0, 32), (32, 0), (64, 96), (96, 64)):
                        nc.gpsimd.dma_start(out=rotq[db:db + 32, :],
                                            in_=qraw[sb:sb + 32, :])
                    t2q = tpool.tile([128, 512], BF, name="t2q", tag="t2q")
                    t3q = tpool.tile([128, 512], BF, name="t3q", tag="t3q")
                    nc.vector.tensor_mul(t2q[:, :], qraw[:, :],
                                         crep[:, cs:cs + 512])
                    nc.vector.tensor_mul(t3q[:, :], rotq[:, :],
                                         ssig[:, cs:cs + 512])
                    # scatter the two heads into qT4 (cols strided by 512)
                    qv = qT4.rearrange("p (blk h r) -> p blk h r", h=4, r=128)
                    for ph in range(2):
                        h = 2 * hp + ph
                        dst = qv[:, b * 16 + 4 * g:b * 16 + 4 * g + 4, h, :]
                        s2 = t2q[ph * 64:(ph + 1) * 64, :].rearrange(
                            "p (j r) -> p j r", r=128)
                        s3 = t3q[ph * 64:(ph + 1) * 64, :].rearrange(
                            "p (j r) -> p j r", r=128)
                        nc.vector.tensor_add(dst, s2, s3)

            # ---- attention (causal, per (b, group-of-4-qbs)) ----
            def emit_pv(b, g, kb, scs, pvt):
                vbase = (b * 16 + kb) * 128
                for (pair, pr) in scs:
                    for jj, qb in enumerate(pair):
                        j = qb - 4 * g
                        nc.tensor.matmul(
                            pvt[j][0:VROW, :],
                            lhsT=vOnes[:, vbase:vbase + VROW],
                            rhs=pr[:, jj * 512:(jj + 1) * 512],
                            start=(kb == 0), stop=(kb == qb))
                        if kb == qb:
                            emit_norm(b, qb, pvt[j])

            def emit_norm(b, qb, pv):
                rsum = apool.tile([1, 512], F32, name="rsum", tag="rsum",
                                  bufs=2)
                nc.vector.tensor_copy(out=rsum[0:1, :], in_=pv[64:65, :])
                rec = apool.tile([1, 512], F32, name="rec", tag="rec", bufs=2)
                nc.vector.reciprocal(out=rec[0:1, :], in_=rsum[0:1, :])
                rep = apool.tile([128, 512], F32, name="rep", tag="rep",
                                 bufs=2)
                nc.gpsimd.partition_broadcast(rep[:, :], rec[0:1, :])
                for t in range(2):
                    for ph in range(2):
                        h = 2 * t + ph
                        nc.vector.tensor_mul(
                            attnT[t][ph * 64:(ph + 1) * 64,
                                     b * S + qb * 128:b * S + (qb + 1) * 128],
                            pv[0:64, h * 128:(h + 1) * 128],
                            rep[ph * 64:(ph + 1) * 64, h * 128:(h + 1) * 128])

            for b in range(B):
                for g in range(4):
                    pvt = [pspool.tile([VROW, 512], F32, name=f"pv{j}",
                                       tag=f"pv{j}") for j in range(4)]
                    pend = None
                    for kb in range(4 * g + 4):
                        active = [qb for qb in range(4 * g, 4 * g + 4)
                                  if qb >= kb]
                        ksl = kT[0:64, b * S + kb * 128:b * S + (kb + 1) * 128]
                        scs = []
                        for ii in range(0, len(active), 2):
                            pair = active[ii:ii + 2]
                            sct = pspool.tile([128, 1024], F32, name="sc",
                                              tag="sc", bufs=2)
                            for jj, qb in enumerate(pair):
                                qs = (b * 16 + qb) * 512
                                nc.tensor.matmul(
                                    sct[:, jj * 512:(jj + 1) * 512],
                                    lhsT=ksl, rhs=qT4[0:64, qs:qs + 512],
                                    start=True, stop=True)
                            w = 512 * len(pair)
                            pr = apool.tile([128, 1024], BF, name="probs",
                                            tag="probs", bufs=6)
                            nc.scalar.activation(out=pr[:, 0:w],
                                                 in_=sct[:, 0:w],
                                                 func=EXP, scale=0.125)
                            scs.append((pair, pr))
                        if active and active[0] == kb:
                            pr0 = scs[0][1]
                            nc.vector.tensor_mul(pr0[:, 0:512], pr0[:, 0:512],
                                                 dmask[:, :])
                        if pend is not None:
                            emit_pv(b, g, pend[0], pend[1], pvt)
                        pend = (kb, scs)
                    emit_pv(b, g, pend[0], pend[1], pvt)

            if dbg:
                nc.sync.dma_start(out=kT_dbg[:, :], in_=kT[0:64, :])
                nc.sync.dma_start(out=vO_dbg[:, :], in_=vOnes[:, :])
                nc.sync.dma_start(out=qT_dbg[:, :], in_=qT4[0:64, :])
                for t in range(2):
                    nc.sync.dma_start(out=aT_dbg[t, :, :], in_=attnT[t][:, :])

            # ---- output projection (partial: this core's 4 heads) ----
            for rb in range(32):
                po = [pspool.tile([128, 512], F32, name=f"po{dc}",
                                  tag=f"pv{dc}") for dc in range(4)]
                for t in range(2):
                    for dc in range(4):
                        nc.tensor.matmul(
                            po[dc][:, :],
                            lhsT=attnT[t][:, rb * 128:(rb + 1) * 128],
                            rhs=wot[t][:, dc * 512:(dc + 1) * 512],
                            start=(t == 0), stop=(t == 1))
                for dc in range(4):
                    ob = tpool.tile([128, 512], BF, name="ob", tag="ob",
                                    bufs=4)
                    if dc % 2 == 0:
                        nc.vector.tensor_copy(out=ob[:, :], in_=po[dc][:, :])
                    else:
                        nc.scalar.copy(out=ob[:, :], in_=po[dc][:, :])
                    eng = nc.sync if dc % 2 == 0 else nc.gpsimd
                    eng.dma_start(
                        out=out_d[rb * 128:(rb + 1) * 128,
                                  dc * 512:(dc + 1) * 512],
                        in_=ob[:, :])

    nc.compile()
    return nc


# --------------------------------------------------------------------------
# host-side sharding / layout prep
# --------------------------------------------------------------------------

def _deint(h):
    """de-interleaved column indices for head h (64 cols: evens then odds)."""
    return h * HD + np.concatenate([np.arange(0, HD, 2), np.arange(1, HD, 2)])


def _prep_shared(x, freqs_cos, freqs_sin, mask):
    xT = np.ascontiguousarray(
        x.reshape(BS, D).T).astype(BF16)                   # [D, BS]
    j = np.arange(128) % 32
    crep1 = freqs_cos[:, j].T                              # [128, S]
    crep = np.tile(crep1, (1, B)).astype(BF16)             # [128, BS]
    sgn = np.where((np.arange(128) // 32) % 2 == 0, -1.0, 1.0).astype(
        np.float32)
    ssig1 = freqs_sin[:, j].T * sgn[:, None]
    ssig = np.tile(ssig1, (1, B)).astype(BF16)
    dm = np.exp(mask[0:128, 0:128]).T                      # [128k, 128q]
    dmask = np.tile(dm, (1, 4)).astype(BF16)               # [128, 512]
    return xT, crep, ssig, dmask


def _prep_core(c, wq, wk, wv, wo):
    heads = [4 * c + h for h in range(4)]
    # wq: [16, 2, 128, 128] (k-tile, head-pair, kdim, deint head cols)
    qcols = np.concatenate([_deint(h) for h in heads])     # [256]
    wq_c = wq[:, qcols].reshape(16, 128, 2, 128).transpose(0, 2, 1, 3)
    wq_c = np.ascontiguousarray(wq_c).astype(BF16)
    # wkv: [16, 128, 128] = [wk deint 64 | wv natural 64]
    kcols = _deint(c)
    vcols = c * HD + np.arange(HD)
    wkv = np.concatenate([wk[:, kcols], wv[:, vcols]], axis=1)  # [D, 128]
    wkv_c = np.ascontiguousarray(wkv.reshape(16, 128, 128)).astype(BF16)
    # wo: [2, 128, D] rows = heads 2t, 2t+1 natural hd
    worows = np.concatenate([h * HD + np.arange(HD) for h in heads])
    wo_c = np.ascontiguousarray(wo[worows, :].reshape(2, 128, D)).astype(BF16)
    return wq_c, wkv_c, wo_c


def _assemble(results):
    acc = np.zeros((BS, D), np.float32)
    for i in range(NCORES):
        acc += np.asarray(results[i]["out"], dtype=np.float32)
    return np.ascontiguousarray(acc.reshape(B, S, D))


LAST_RUN_INFO = {}


def kernel(x, freqs_cos, freqs_sin, mask, wq, wk, wv, wo, start_pos=0):
    from concourse.bass_utils import run_bass_kernel_spmd

    x = np.asarray(x, dtype=np.float32)
    freqs_cos = np.asarray(freqs_cos, dtype=np.float32)
    freqs_sin = np.asarray(freqs_sin, dtype=np.float32)
    mask = np.asarray(mask, dtype=np.float32)
    wq = np.asarray(wq, dtype=np.float32)
    wk = np.asarray(wk, dtype=np.float32)
    wv = np.asarray(wv, dtype=np.float32)
    wo = np.asarray(wo, dtype=np.float32)

    xT, crep, ssig, dmask = _prep_shared(x, freqs_cos, freqs_sin, mask)
    in_maps = []
    for c in range(NCORES):
        wq_c, wkv_c, wo_c = _prep_core(c, wq, wk, wv, wo)
        in_maps.append({
            "xT": xT, "wq": wq_c, "wkv": wkv_c, "wo": wo_c,
            "crep": crep, "ssig": ssig, "dmask": dmask,
        })

    nc = _build_nc()

    trace = bool(int(os.environ.get("KERNEL_TRACE", "0")))
    kwargs = {}
    if trace:
        _install_ntff_hook()
        import concourse.bass_utils as bass_utils
        bass_utils.upload_artifacts = lambda tmpdir: tmpdir
        import tempfile
        tmpdir = tempfile.mkdtemp(prefix="attn_trace_")
        kwargs = {"trace": True, "tmpdir": tmpdir}

    res = run_bass_kernel_spmd(nc, in_maps, core_ids=list(range(NCORES)),
                               **kwargs)
    LAST_RUN_INFO.clear()
    LAST_RUN_INFO.update({
        "exec_time_ns": res.exec_time_ns,
        "tmpdir": kwargs.get("tmpdir"),
        "res": res,
    })
    return _assemble(res.results)


def _install_ntff_hook():
    if "antenv.axon_hooks" not in sys.modules:
        import antenv

        mod = types.ModuleType("antenv.axon_hooks")
        mod._hook = None
        mod.set_axon_ntff_profile_hook = lambda h: setattr(mod, "_hook", h)
        mod.get_axon_ntff_profile_hook = lambda: mod._hook
        sys.modules["antenv.axon_hooks"] = mod
        antenv.axon_hooks = mod
    from trn_agent_boot.trn_boot import _ntff_profile_via_ctypes
    from antenv.axon_hooks import set_axon_ntff_profile_hook as _set

    _set(_ntff_profile_via_ctypes("/opt/axon/libaxon_pjrt.so"))


# revision 10
# speedup vs baseline: 1.6339x; 1.5052x over previous
"""Distributed GQA attention kernel for 8 TRN2 NeuronCores.

Problem: B=2, S=2048, D=2048, 32 q-heads / 8 kv-heads, hd=64, causal + RoPE.

Strategy (kv-head tensor parallel, zero collectives):
  - Core c owns kv-head c (q-heads 4c..4c+3) for BOTH batches over ALL rows.
    Every core loads the full x (host-pretransposed to xT bf16) and projects
    Q (4 heads), K, V (1 kv head each) for all 4096 rows. K/V never leave the
    core, so there are NO collectives. Each core computes a PARTIAL output
    (its 4 heads x its 256 wo rows) and the host sums the 8 partials.
  - Because every core sees all rows, the causal structure is IDENTICAL on
    all cores (SPMD-compatible): per q-block qb only key blocks kb <= qb are
    computed -> ~47% of score/exp/PV work skipped exactly, with all matmuls
    staying N=512 wide (4 q-heads x 128 rows share one kv head -> one ksl
    stationary serves 4 heads; one vsl serves all q-blocks at a kb).
  - Attention runs fully "transposed": scoresT = ksl.T @ qT4 with keys on
    partitions; exp is merged into [128,1024] two-bank PSUM reads (one ACT
    instruction per 2 score tiles); only the diagonal tile gets a mask
    multiply. PV uses V in natural layout (via DMA-transpose from the
    projection) with a ones-column appended for the softmax denominator.
  - Softmax without max-subtraction: probs = exp(s/8); denominator from the
    ones-column; normalization applied to the PV output via
    reciprocal + partition_broadcast + elementwise multiply.
  - Matmuls in bf16; psums/softmax in fp32; partial output stored bf16.

kernel(**inputs) -> np.ndarray  takes full inputs, returns full [2,2048,2048].
"""

import functools
import os
import sys
import types

import numpy as np
import ml_dtypes

BF16 = ml_dtypes.bfloat16

B, S, D = 2, 2048, 2048
NH, NKV, HD = 32, 8, 64
BS = B * S               # 4096 rows total (b-major)
NB = S // 128            # 16 blocks per batch
NCORES = 8
VROW = HD + 1            # 65: [v | 1]


# --------------------------------------------------------------------------
# device graph (identical on all cores; per-core weights via input data)
# --------------------------------------------------------------------------

@functools.lru_cache(maxsize=None)
def _build_nc():
    import concourse.bacc as bacc
    import concourse.mybir as mybir
    import concourse.tile as tile

    BF = mybir.dt.bfloat16
    F32 = mybir.dt.float32
    EXP = mybir.ActivationFunctionType.Exp

    nc = bacc.Bacc(trn_type="TRN2", target_bir_lowering=False, debug=False,
                   num_devices=NCORES)

    dbg = bool(int(os.environ.get("KERNEL_DEBUG", "0")))
    if dbg:
        kT_dbg = nc.declare_dram_parameter("kT_dbg", [64, BS], BF,
                                           isOutput=True)
        vO_dbg = nc.declare_dram_parameter("vO_dbg", [128, 32 * 128], BF,
                                           isOutput=True)
        qT_dbg = nc.declare_dram_parameter("qT_dbg", [64, 32 * 512], BF,
                                           isOutput=True)
        aT_dbg = nc.declare_dram_parameter("aT_dbg", [2, 128, BS], BF,
                                           isOutput=True)

    xT_d = nc.declare_dram_parameter("xT", [D, BS], BF, isOutput=False)
    wq_d = nc.declare_dram_parameter("wq", [2, 128, 2048], BF, isOutput=False)
    wkv_d = nc.declare_dram_parameter("wkv", [128, 2048], BF, isOutput=False)
    wo_d = nc.declare_dram_parameter("wo", [2, 128, D], BF, isOutput=False)
    crep_d = nc.declare_dram_parameter("crep", [128, BS], BF, isOutput=False)
    ssig_d = nc.declare_dram_parameter("ssig", [128, BS], BF, isOutput=False)
    dmask_d = nc.declare_dram_parameter("dmask", [128, 512], BF, isOutput=False)
    out_d = nc.declare_dram_parameter("out", [BS, D], BF, isOutput=True)

    with tile.TileContext(nc) as tc:
        with tc.tile_pool(name="const", bufs=1) as cpool, \
             tc.tile_pool(name="persist", bufs=1) as ppool, \
             tc.tile_pool(name="xstream", bufs=32) as xpool, \
             tc.tile_pool(name="wstream", bufs=4) as wpool, \
             tc.tile_pool(name="work", bufs=3) as tpool, \
             tc.tile_pool(name="attn", bufs=3) as apool, \
             tc.tile_pool(name="ps", bufs=1, space="PSUM") as pspool:

            # ---- constants ----
            crep = cpool.tile([128, BS], BF, name="crep", tag="crep")
            nc.sync.dma_start(out=crep[:, :], in_=crep_d[:, :])
            ssig = cpool.tile([128, BS], BF, name="ssig", tag="ssig")
            nc.sync.dma_start(out=ssig[:, :], in_=ssig_d[:, :])
            dmask = cpool.tile([128, 512], BF, name="dmask", tag="dmask")
            nc.sync.dma_start(out=dmask[:, :], in_=dmask_d[:, :])
            wot = []
            for t in range(2):
                w_ = cpool.tile([128, D], BF, name=f"wot{t}", tag=f"wot{t}")
                nc.gpsimd.dma_start(out=w_[:, :], in_=wo_d[t, :, :])
                wot.append(w_)
            # resident projection weights (tiny: 2.5MB total)
            wkv_sb = cpool.tile([128, 2048], BF, name="wkv_sb", tag="wkv_sb")
            nc.sync.dma_start(out=wkv_sb[:, :], in_=wkv_d[:, :])
            wq_sb = []
            for hp in range(2):
                w_ = cpool.tile([128, 2048], BF, name=f"wq_sb{hp}",
                                tag=f"wq_sb{hp}")
                nc.sync.dma_start(out=w_[:, :], in_=wq_d[hp, :, :])
                wq_sb.append(w_)

            # ---- persistent activations ----
            kT = ppool.tile([64, BS], BF, name="kT", tag="kT")
            # V blocks padded to 128-col stride: dma_start_transpose needs
            # 128-byte-aligned output offsets (65-col stride corrupts).
            vOnes = ppool.tile([128, 32 * 128], BF, name="vOnes", tag="vOnes")
            vview = vOnes.rearrange("p (n w) -> p n w", w=128)
            nc.gpsimd.memset(vview[:, :, HD:HD + 1], 1.0)
            # qT4 cols: (b*16 + qb)*512 + h*128 + r   (h = head 0..3 local)
            qT4 = ppool.tile([64, 32 * 512], BF, name="qT4", tag="qT4")
            attnT = []
            for t in range(2):
                a_ = ppool.tile([128, BS], BF, name=f"attnT{t}", tag=f"attnT{t}")
                attnT.append(a_)

            # ---- projections (8 chunks of 512 rows; c = b*4 + g) ----
            kvAll = ppool.tile([128, BS], BF, name="kvAll", tag="kvAll")
            qAll = []
            for hp in range(2):
                q_ = ppool.tile([128, BS], BF, name=f"qAll{hp}",
                                tag=f"qAll{hp}")
                qAll.append(q_)
            for c in range(8):
                cs = c * 512
                xt = []
                for k in range(16):
                    t_ = xpool.tile([128, 512], BF, name="xt", tag="xt")
                    eng = nc.sync if k % 2 == 0 else nc.gpsimd
                    eng.dma_start(out=t_[:, :],
                                  in_=xT_d[k * 128:(k + 1) * 128, cs:cs + 512])
                    xt.append(t_)

                # K|V fused projection: psum rows 0:64 = kT, 64:128 = vT
                ps = pspool.tile([128, 512], F32, name="pskv", tag=f"pv{c % 4}")
                for k in range(16):
                    nc.tensor.matmul(ps[:, :],
                                     lhsT=wkv_sb[:, k * 128:(k + 1) * 128],
                                     rhs=xt[k][:, :],
                                     start=(k == 0), stop=(k == 15))
                nc.vector.tensor_copy(out=kvAll[:, cs:cs + 512], in_=ps[:, :])

                # Q projection
                for hp in range(2):
                    psq = pspool.tile([128, 512], F32, name="psq",
                                      tag=f"pv{(c + 2 + hp) % 4}")
                    for k in range(16):
                        nc.tensor.matmul(psq[:, :],
                                         lhsT=wq_sb[hp][:, k * 128:(k + 1) * 128],
                                         rhs=xt[k][:, :],
                                         start=(k == 0), stop=(k == 15))
                    if hp == 0:
                        nc.vector.tensor_copy(out=qAll[hp][:, cs:cs + 512],
                                              in_=psq[:, :])
                    else:
                        nc.scalar.copy(out=qAll[hp][:, cs:cs + 512],
                                       in_=psq[:, :])

            # ---- RoPE (wide, once) + V transposes ----
            # K: rows 0:64 of kvAll
            rotk = tpool.tile([64, BS], BF, name="rotk", tag="rotk", bufs=1)
            nc.gpsimd.dma_start(out=rotk[0:32, :], in_=kvAll[32:64, :])
            nc.sync.dma_start(out=rotk[32:64, :], in_=kvAll[0:32, :])
            nc.vector.tensor_mul(kT[0:64, :], kvAll[0:64, :], crep[0:64, :])
            nc.vector.tensor_mul(rotk[0:64, :], rotk[0:64, :], ssig[0:64, :])
            nc.vector.tensor_add(kT[0:64, :], kT[0:64, :], rotk[0:64, :])
            # V natural layout via DMA transpose (rows 64:128 of kvAll)
            for blk in range(32):
                nc.scalar.dma_start_transpose(
                    out=vOnes[:, blk * 128:blk * 128 + HD],
                    in_=kvAll[64:128, blk * 128:(blk + 1) * 128])
            # Q: per head-pair tile (in-place muls to save SBUF)
            qv = qT4.rearrange("p (blk h r) -> p blk h r", h=4, r=128)
            for hp in range(2):
                rotq = tpool.tile([128, BS], BF, name="rotq", tag="rotq",
                                  bufs=1)
                for i, (db, sb) in enumerate(
                        ((0, 32), (32, 0), (64, 96), (96, 64))):
                    eng = nc.sync if i % 2 == 0 else nc.gpsimd
                    eng.dma_start(out=rotq[db:db + 32, :],
                                  in_=qAll[hp][sb:sb + 32, :])
                nc.vector.tensor_mul(qAll[hp][:, :], qAll[hp][:, :],
                                     crep[:, :])
                nc.vector.tensor_mul(rotq[:, :], rotq[:, :], ssig[:, :])
                for ph in range(2):
                    h = 2 * hp + ph
                    dst = qv[:, :, h, :]
                    s2 = qAll[hp][ph * 64:(ph + 1) * 64, :].rearrange(
                        "p (j r) -> p j r", r=128)
                    s3 = rotq[ph * 64:(ph + 1) * 64, :].rearrange(
                        "p (j r) -> p j r", r=128)
                    nc.vector.tensor_add(dst, s2, s3)

            # ---- attention (causal, per (b, group-of-4-qbs)) ----
            def emit_pv(b, g, kb, scs, pvt):
                vbase = (b * 16 + kb) * 128
                for (pair, pr) in scs:
                    for jj, qb in enumerate(pair):
                        j = qb - 4 * g
                        nc.tensor.matmul(
                            pvt[j][0:VROW, :],
                            lhsT=vOnes[:, vbase:vbase + VROW],
                            rhs=pr[:, jj * 512:(jj + 1) * 512],
                            start=(kb == 0), stop=(kb == qb))
                        if kb == qb:
                            emit_norm(b, qb, pvt[j])

            def emit_norm(b, qb, pv):
                rsum = apool.tile([1, 512], F32, name="rsum", tag="rsum",
                                  bufs=1)
                nc.vector.tensor_copy(out=rsum[0:1, :], in_=pv[64:65, :])
                rsb = apool.tile([128, 512], F32, name="rsb", tag="rsb",
                                 bufs=1)
                nc.gpsimd.partition_broadcast(rsb[:, :], rsum[0:1, :])
                rep = apool.tile([128, 512], F32, name="rep", tag="rep",
                                 bufs=1)
                nc.vector.reciprocal(out=rep[:, :], in_=rsb[:, :])
                for t in range(2):
                    for ph in range(2):
                        h = 2 * t + ph
                        nc.vector.tensor_mul(
                            attnT[t][ph * 64:(ph + 1) * 64,
                                     b * S + qb * 128:b * S + (qb + 1) * 128],
                            pv[0:64, h * 128:(h + 1) * 128],
                            rep[ph * 64:(ph + 1) * 64, h * 128:(h + 1) * 128])

            for b in range(B):
                for g in range(4):
                    pvt = [pspool.tile([VROW, 512], F32, name=f"pv{j}",
                                       tag=f"pv{j}") for j in range(4)]
                    pend = None
                    for kb in range(4 * g + 4):
                        active = [qb for qb in range(4 * g, 4 * g + 4)
                                  if qb >= kb]
                        ksl = kT[0:64, b * S + kb * 128:b * S + (kb + 1) * 128]
                        scs = []
                        for ii in range(0, len(active), 2):
                            pair = active[ii:ii + 2]
                            sct = pspool.tile([128, 1024], F32, name="sc",
                                              tag="sc", bufs=2)
                            for jj, qb in enumerate(pair):
                                qs = (b * 16 + qb) * 512
                                nc.tensor.matmul(
                                    sct[:, jj * 512:(jj + 1) * 512],
                                    lhsT=ksl, rhs=qT4[0:64, qs:qs + 512],
                                    start=True, stop=True)
                            w = 512 * len(pair)
                            pr = apool.tile([128, 1024], BF, name="probs",
                                            tag="probs", bufs=6)
                            nc.scalar.activation(out=pr[:, 0:w],
                                                 in_=sct[:, 0:w],
                                                 func=EXP, scale=0.125)
                            scs.append((pair, pr))
                        if active and active[0] == kb:
                            pr0 = scs[0][1]
                            nc.vector.tensor_mul(pr0[:, 0:512], pr0[:, 0:512],
                                                 dmask[:, :])
                        if pend is not None:
                            emit_pv(b, g, pend[0], pend[1], pvt)
                        pend = (kb, scs)
                    emit_pv(b, g, pend[0], pend[1], pvt)

            if dbg:
                nc.sync.dma_start(out=kT_dbg[:, :], in_=kT[0:64, :])
                nc.sync.dma_start(out=vO_dbg[:, :], in_=vOnes[:, :])
                nc.sync.dma_start(out=qT_dbg[:, :], in_=qT4[0:64, :])
                for t in range(2):
                    nc.sync.dma_start(out=aT_dbg[t, :, :], in_=attnT[t][:, :])

            # ---- output projection (partial: this core's 4 heads) ----
            for rb in range(32):
                po = [pspool.tile([128, 512], F32, name=f"po{dc}",
                                  tag=f"pv{dc}") for dc in range(4)]
                for t in range(2):
                    for dc in range(4):
                        nc.tensor.matmul(
                            po[dc][:, :],
                            lhsT=attnT[t][:, rb * 128:(rb + 1) * 128],
                            rhs=wot[t][:, dc * 512:(dc + 1) * 512],
                            start=(t == 0), stop=(t == 1))
                for dc in range(4):
                    ob = tpool.tile([128, 512], BF, name="ob", tag="ob",
                                    bufs=4)
                    if dc % 2 == 0:
                        nc.vector.tensor_copy(out=ob[:, :], in_=po[dc][:, :])
                    else:
                        nc.scalar.copy(out=ob[:, :], in_=po[dc][:, :])
                    eng = nc.sync if dc % 2 == 0 else nc.gpsimd
                    eng.dma_start(
                        out=out_d[rb * 128:(rb + 1) * 128,
                                  dc * 512:(dc + 1) * 512],
                        in_=ob[:, :])

    nc.compile()
    return nc


# --------------------------------------------------------------------------
# host-side sharding / layout prep
# --------------------------------------------------------------------------

def _deint(h):
    """de-interleaved column indices for head h (64 cols: evens then odds)."""
    return h * HD + np.concatenate([np.arange(0, HD, 2), np.arange(1, HD, 2)])


def _prep_shared(x, freqs_cos, freqs_sin, mask):
    xT = np.ascontiguousarray(
        x.reshape(BS, D).T).astype(BF16)                   # [D, BS]
    j = np.arange(128) % 32
    crep1 = freqs_cos[:, j].T                              # [128, S]
    crep = np.tile(crep1, (1, B)).astype(BF16)             # [128, BS]
    sgn = np.where((np.arange(128) // 32) % 2 == 0, -1.0, 1.0).astype(
        np.float32)
    ssig1 = freqs_sin[:, j].T * sgn[:, None]
    ssig = np.tile(ssig1, (1, B)).astype(BF16)
    dm = np.exp(mask[0:128, 0:128]).T                      # [128k, 128q]
    dmask = np.tile(dm, (1, 4)).astype(BF16)               # [128, 512]
    return xT, crep, ssig, dmask


def _prep_core(c, wq, wk, wv, wo):
    heads = [4 * c + h for h in range(4)]
    # wq: [2, 128, 2048] (head-pair, kdim-within-tile, k-tile*128 + head col)
    qcols = np.concatenate([_deint(h) for h in heads])     # [256]
    wq_c = wq[:, qcols].reshape(16, 128, 2, 128).transpose(2, 1, 0, 3)
    wq_c = np.ascontiguousarray(wq_c.reshape(2, 128, 2048)).astype(BF16)
    # wkv: [128, 2048] = per k-tile 128 cols: [wk deint 64 | wv natural 64]
    kcols = _deint(c)
    vcols = c * HD + np.arange(HD)
    wkv = np.concatenate([wk[:, kcols], wv[:, vcols]], axis=1)  # [D, 128]
    wkv_c = wkv.reshape(16, 128, 128).transpose(1, 0, 2)
    wkv_c = np.ascontiguousarray(wkv_c.reshape(128, 2048)).astype(BF16)
    # wo: [2, 128, D] rows = heads 2t, 2t+1 natural hd
    worows = np.concatenate([h * HD + np.arange(HD) for h in heads])
    wo_c = np.ascontiguousarray(wo[worows, :].reshape(2, 128, D)).astype(BF16)
    return wq_c, wkv_c, wo_c


def _assemble(results):
    acc = np.zeros((BS, D), np.float32)
    for i in range(NCORES):
        acc += np.asarray(results[i]["out"], dtype=np.float32)
    return np.ascontiguousarray(acc.reshape(B, S, D))


LAST_RUN_INFO = {}


def kernel(x, freqs_cos, freqs_sin, mask, wq, wk, wv, wo, start_pos=0):
    from concourse.bass_utils import run_bass_kernel_spmd

    x = np.asarray(x, dtype=np.float32)
    freqs_cos = np.asarray(freqs_cos, dtype=np.float32)
    freqs_sin = np.asarray(freqs_sin, dtype=np.float32)
    mask = np.asarray(mask, dtype=np.float32)
    wq = np.asarray(wq, dtype=np.float32)
    wk = np.asarray(wk, dtype=np.float32)
    wv = np.asarray(wv, dtype=np.float32)
    wo = np.asarray(wo, dtype=np.float32)

    xT, crep, ssig, dmask = _prep_shared(x, freqs_cos, freqs_sin, mask)
    in_maps = []
    for c in range(NCORES):
        wq_c, wkv_c, wo_c = _prep_core(c, wq, wk, wv, wo)
        in_maps.append({
            "xT": xT, "wq": wq_c, "wkv": wkv_c, "wo": wo_c,
            "crep": crep, "ssig": ssig, "dmask": dmask,
        })

    nc = _build_nc()

    trace = bool(int(os.environ.get("KERNEL_TRACE", "0")))
    kwargs = {}
    if trace:
        _install_ntff_hook()
        import concourse.bass_utils as bass_utils
        bass_utils.upload_artifacts = lambda tmpdir: tmpdir
        import tempfile
        tmpdir = tempfile.mkdtemp(prefix="attn_trace_")
        kwargs = {"trace": True, "tmpdir": tmpdir}

    res = run_bass_kernel_spmd(nc, in_maps, core_ids=list(range(NCORES)),
                               **kwargs)
    LAST_RUN_INFO.clear()
    LAST_RUN_INFO.update({
        "exec_time_ns": res.exec_time_ns,
        "tmpdir": kwargs.get("tmpdir"),
        "res": res,
    })
    return _assemble(res.results)


def _install_ntff_hook():
    if "antenv.axon_hooks" not in sys.modules:
        import antenv

        mod = types.ModuleType("antenv.axon_hooks")
        mod._hook = None
        mod.set_axon_ntff_profile_hook = lambda h: setattr(mod, "_hook", h)
        mod.get_axon_ntff_profile_hook = lambda: mod._hook
        sys.modules["antenv.axon_hooks"] = mod
        antenv.axon_hooks = mod
    from trn_agent_boot.trn_boot import _ntff_profile_via_ctypes
    from antenv.axon_hooks import set_axon_ntff_profile_hook as _set

    _set(_ntff_profile_via_ctypes("/opt/axon/libaxon_pjrt.so"))
